# revision 25
# baseline (speedup 1.0000x reference)
"""AlignmentContrastiveLoss Trainium2 kernel (v10).

Math (matching the reference):
  im = im_set[:, 1:, :]        -> [128, 64, 1024]  rows (b, i)
  s  = s_seq[:, 1:-2, :]       -> [128, 64, 1024]  rows (t, j)
  align[b,t,i,j] = im[b,i,:] . s[t,j,:]   (masked entries forced to 0)
  aggr[b,t] = sum_j max_i align
  loss = hinge-contrastive reduction of aggr [128,128]

Structure:
  - Work split 4 word-shards x 2 image-column-shards across 8 cores.
    Valid words only (bin-packed); valid image regions only, padded per
    slot to a shared two-shard template width (multiple of 4).
  - fp8e4 DoubleRow matmuls (contraction 256/instruction, ~157 TF/s).
  - Chunk-contiguous HBM layout: each PSUM-chunk's [KT x ccols] block is
    contiguous per partition, so a chunk DMA is 128 descriptors of up to
    4KB instead of 1024 x ~500B.
  - All DMAs are issued up front on ONE HWDGE queue (SP) in priority
    order (splitting across two queues halves each queue's share of the
    16 DMA engines). The first two chunks transfer in ko-halves so the
    first matmuls gate on quarter-size pieces.
  - Tau-major over two chunk groups: group 0 = the two smallest chunks
    (starts as soon as ~250KB has landed), group 1 = the rest. Within a
    tau, each chunk runs its 4 ko-matmuls back-to-back and immediately
    emits its segmented max-reduces, so each chunk's PSUM stop lands as
    early as possible and DVE pipelines behind the PE.
  - The device outputs the per-word-row maxes m_all[128, T*BL] (fp16)
    straight from the last reduce; the masked-region floor clamp, the
    j-sum over word rows, and the hinge loss all run on the host. This
    removes the indicator matmuls, clamps, PSUM accumulation bank, and
    final copy from the device critical path.
  - The PE is power-throttled on sustained fp8 work (~50% util-limit
    windows, 7-10us total), so junk warmup matmuls and extra padding
    columns are counter-productive - the schedule minimizes total PE
    cycles and lets the PE idle before data arrives rather than burning
    the power budget.
  - A post-Tile pass prunes/migrates redundant semaphore waits (TPB ISA
    encodes ONE wait per instruction); see _prune_redundant_waits.

Compiled programs are cached per plan signature.
"""

import numpy as np
import ml_dtypes

import concourse.bass as bass
import concourse.mybir as mybir
import concourse.tile as tile
from concourse.bass_utils import run_bass_kernel_spmd

F32 = mybir.dt.float32
F16 = mybir.dt.float16
F8 = mybir.dt.float8e4
NP_F8 = ml_dtypes.float8_e4m3

MARGIN = 0.2
B = 128
LI = 64          # image regions after slicing
LS = 64          # words after slicing
D = 1024
KT = D // 128    # 8 contraction subtiles of 128
NCORES = 8
CHUNK_MAX = 512  # PSUM bank width (fp32)
DR = mybir.MatmulPerfMode.DoubleRow

N_WS = 4    # word (sentence) shards
N_CS = 2    # image-column shards; core id = ws * N_CS + cs
BL = B // N_CS  # b's per column shard

N_WARM = 0       # junk warmup matmuls burn power budget -> more PE throttling
POOL_REDUCE = 0  # gpsimd tensor_reduce can't do free-dim (X) reduces on TRN2


class _Plan:
    __slots__ = (
        "im_l", "s_l", "shard_bs", "template", "chunks", "n_tot",
        "t_tiles", "ns", "core_sents", "signature",
    )


def _make_plan(im_len, s_len):
    p = _Plan()
    im_l = np.asarray(im_len).astype(np.int64) - 1
    s_l = np.asarray(s_len).astype(np.int64) - 3
    im_l = np.clip(im_l, 0, LI)
    s_l = np.clip(s_l, 0, LS)
    p.im_l = im_l
    p.s_l = s_l

    # --- image columns ---------------------------------------------------
    # width = valid count padded up to a multiple of 4. No zero-guard
    # column: the reference's "max includes 0 when any i is masked" is
    # reproduced by a post-reduce clamp against a per-slot floor.
    widths = np.maximum(4, 4 * ((im_l + 3) // 4)).astype(np.int64)
    order = np.argsort(-widths, kind="stable")
    shard_bs = [
        [int(order[j * N_CS + k]) for j in range(BL)] for k in range(N_CS)
    ]
    template = [int(widths[order[j * N_CS]]) for j in range(BL)]
    p.shard_bs = shard_bs
    p.template = template

    # chunks over the template (shared by all cores): greedy <= CHUNK_MAX
    # columns, breaks only at slot boundaries; runs = maximal equal-width
    # slot ranges for the segmented reduces.
    chunks = []  # [cols, col_off, runs, slot0, nslots]
    i = 0
    while i < BL:
        cols = 0
        runs = []
        s0 = i
        while i < BL:
            w = template[i]
            if cols + w > CHUNK_MAX:
                break
            j = i
            while (
                j < BL
                and template[j] == w
                and cols + (j - i + 1) * w <= CHUNK_MAX
            ):
                j += 1
            runs.append((w, j - i, cols, i))
            cols += (j - i) * w
            i = j
        chunks.append([cols, 0, tuple(runs), s0, i - s0])
    # traversal order: the two smallest chunks first (group 0 — they gate
    # the first real matmuls), then the rest descending with the smallest
    # of them last so the final reduce+j-sum tail is short.
    chunks.sort(key=lambda c: c[0])
    if len(chunks) > 3:
        head, rest = chunks[:2], chunks[2:]
        tail = rest.pop(0)  # smallest of rest
        chunks = head + rest[::-1] + [tail]
    col = 0
    for c in chunks:
        c[1] = col
        col += c[0]
    chunks = [tuple(c) for c in chunks]
    p.chunks = chunks
    p.n_tot = col

    # --- sentence packing: greedy bin-pack into N_WS word shards --------
    order_s = np.argsort(-s_l, kind="stable")
    loads = [0] * N_WS
    core_sents = [[] for _ in range(N_WS)]
    for t in order_s:
        c = int(np.argmin(loads))
        core_sents[c].append(int(t))
        loads[c] += int(s_l[t])
    p.core_sents = core_sents
    p.t_tiles = max(1, int(-(-max(loads) // 128)))
    max_ns = max(len(cs) for cs in core_sents)
    p.ns = -(-max_ns // 8) * 8  # pad to multiple of 8

    p.signature = (
        p.t_tiles, p.ns,
        tuple((c[0], c[1], c[3], c[4]) for c in chunks),
    )
    return p


# --------------------------------------------------------------------------
# Device program
# --------------------------------------------------------------------------

def _build_nc(t_tiles, ns, chunks, n_tot,
              prune=True, detect_races=True,
              n_warm=N_WARM, pool_reduce=POOL_REDUCE):
    from contextlib import ExitStack

    T = t_tiles
    nc = bass.Bass(detect_race_conditions=detect_races)
    sT_in = nc.dram_tensor("s_t", [128, T * KT * 128], F8, kind="ExternalInput")
    im_in = nc.dram_tensor("im_pk", [128, KT * n_tot], F8, kind="ExternalInput")
    m_out = nc.dram_tensor("m_out", [128, T * BL], F16, kind="ExternalOutput")

    NCH = len(chunks)
    with tile.TileContext(nc) as tc, ExitStack() as ctx:
        consts = ctx.enter_context(tc.tile_pool(name="consts", bufs=1))
        mp = ctx.enter_context(tc.tile_pool(name="mp", bufs=1))
        imtp = ctx.enter_context(tc.tile_pool(name="imtp", bufs=NCH))
        outp = ctx.enter_context(tc.tile_pool(name="outp", bufs=1))
        psm = ctx.enter_context(tc.tile_pool(name="psm", bufs=7, space="PSUM"))

        # sT[p, tau, k, m] = s_packed[tau*128 + m, k*128 + p]  (fp8)
        sT = consts.tile([128, T, KT, 128], F8)
        # zero tile feeding the warmup matmuls
        zw = consts.tile([128, 2, 256], F8)

        # m_all[p, tau, slot] = max_i of align for word row (tau, p) vs
        # this column shard's slot-th image batch. Clamping against the
        # masked-region floor and the j-sum over word rows both happen on
        # the host - m_all is the kernel's output.
        m_all = mp.tile([128, T, BL], F16)

        # ---- PE warmup: junk matmuls on zeroed SBUF burn the p-state
        # ramp while the first real DMAs are still in flight.
        if n_warm:
            nc.gpsimd.memset(zw[:], 0.0)
            warm = psm.tile([128, 256], F32, tag="pm", name="warm")
            for _ in range(n_warm):
                nc.tensor.matmul(
                    warm[:], zw[:, :, 0:128], zw[:],
                    start=True, stop=True, perf_mode=DR,
                    skip_group_check=True,
                )

        # ---- all DMAs issued up front on a single HWDGE queue (SP), in
        # strict priority order: splitting across two queues halves each
        # queue's share of the 16 DMA engines and delays the startup-
        # critical transfers. The first two chunks are transferred in
        # ko-halves so the first matmuls gate on quarter-size pieces.
        imts = [None] * NCH

        def _issue_chunk(ci, halves=False):
            ccols, coff, runs, s0, nsl = chunks[ci]
            imt = imtp.tile([128, KT, ccols], F8, tag="imt", name=f"im{ci}")
            imts[ci] = imt
            if halves:
                kh = KT // 2
                nc.sync.dma_start(
                    imt[:, 0:kh],
                    im_in[:, KT * coff:KT * coff + kh * ccols].rearrange(
                        "p (k c) -> p k c", k=kh
                    ),
                )
                nc.sync.dma_start(
                    imt[:, kh:KT],
                    im_in[:, KT * coff + kh * ccols:KT * (coff + ccols)]
                    .rearrange("p (k c) -> p k c", k=kh),
                )
            else:
                nc.sync.dma_start(
                    imt[:],
                    im_in[:, KT * coff:KT * (coff + ccols)].rearrange(
                        "p (k c) -> p k c", k=KT
                    ),
                )

        def _issue_sT(ta, tb):
            nc.sync.dma_start(
                sT[:, ta:tb],
                sT_in[:, ta * KT * 128:tb * KT * 128],
            )

        # sT0 rides the Act queue so it transfers in parallel with c0 on
        # the SP queue; Act goes idle right after, returning its DMA
        # engines to the SP queue for the rest of the stream.
        nc.scalar.dma_start(sT[:, 0:1], sT_in[:, 0:KT * 128])
        _issue_chunk(0, halves=True)
        if T > 1:
            nc.scalar.dma_start(sT[:, 1:2], sT_in[:, KT * 128:2 * KT * 128])
        _issue_chunk(1, halves=True)
        if T > 2:
            nc.scalar.dma_start(sT[:, 2:3], sT_in[:, 2 * KT * 128:3 * KT * 128])
        if T > 3:
            _issue_sT(3, min(6, T))
        if NCH > 2:
            _issue_chunk(2)
        if T > 6:
            _issue_sT(6, T)
        for ci in range(3, NCH):
            _issue_chunk(ci)

        # ---- tau-major main loop over two chunk groups.
        groups = [(0, min(2, NCH)), (min(2, NCH), NCH)]
        groups = [(a, b) for a, b in groups if b > a]

        for gi, (clo, chi) in enumerate(groups):
            last_group = gi == len(groups) - 1
            for tau in range(T):
                # per chunk: 4 ko-inner matmuls, then that chunk's
                # reduces — each chunk's PSUM stop lands as early as
                # possible so DVE pipelines behind the PE instead of all
                # reduces bunching after the tau's last matmul.
                for ci in range(clo, chi):
                    ccols, coff, runs, s0, nsl = chunks[ci]
                    pt = psm.tile(
                        [128, ccols], F32, tag="pm", name=f"pm{ci}_{tau}"
                    )
                    for ko in range(KT // 2):
                        nc.tensor.matmul(
                            pt[:],
                            sT[:, tau, 2 * ko:2 * ko + 2, :],
                            imts[ci][:, 2 * ko:2 * ko + 2, :],
                            start=(ko == 0),
                            stop=(ko == KT // 2 - 1),
                            perf_mode=DR,
                        )
                    for (w, nb, roff, slot0) in runs:
                        nc.vector.reduce_max(
                            m_all[:, tau, slot0:slot0 + nb],
                            pt[:, roff:roff + w * nb].rearrange(
                                "p (n w) -> p n w", w=w
                            ),
                            axis=mybir.AxisListType.X,
                        )
        nc.sync.dma_start(
            m_out[:], m_all[:].rearrange("p t s -> p (t s)")
        )

    if prune:
        _prune_redundant_waits(nc)
    return nc


def _prune_redundant_waits(nc):
    """Drop semaphore waits that are provably redundant on the final schedule.

    Walrus's per-instruction ISA structs encode very few sync waits (one for
    PE Matmult / HWDGE DMA), and Tile's wait placement leaves redundant ones:
    (a) waits on the instruction's own processor semaphore (PE matmuls
    complete in program order; a HWDGE queue executes its descriptors FIFO),
    and (b) waits whose target completion is already in the causal past of
    another wait kept on the same instruction. Both classes are dropped here
    using a conservative happens-before computed from the untouched program.

    "Processor" is the engine, except DMACopy where it is the HW queue
    (identified by its update semaphore). Ldweights can be pulled ahead of
    in-flight matmuls by the PE, so it neither extends nor inherits the
    same-proc completion chain.
    """
    insts = []
    for f in nc.m.functions:
        for bb in f.blocks:
            insts.extend(bb.instructions)

    def proc_of(i, idx):
        if i.opcode == "DMACopy":
            ups = i.sync_info.on_update
            qs = [u.ant_name for u in ups if "DMA" in u.ant_name]
            if len(qs) == 1:
                return qs[0]
            return f"__solo_{idx}"
        return f"__eng_{i.engine}"

    # completion clocks: clock[i] = {sem: min guaranteed value when i completes}
    sem_events = {}   # sem -> list of (cumval, inst_idx) in inc order
    sem_cum = {}
    clocks = [None] * len(insts)
    last_in_proc = {}
    # Ldweights waits are satisfied before any later instruction on the
    # engine dispatches (NX evaluates waits in program order; the PE can
    # only pull an LDW *earlier*), so they propagate forward — but LDW
    # itself must not inherit the chain (it may run before prior MMs
    # complete).
    ldw_pending = {}

    def join(a, b):
        for k, v in b.items():
            if a.get(k, -1) < v:
                a[k] = v
        return a

    def producer_clock(sem, val):
        evs = sem_events.get(sem)
        if not evs:
            return None
        # first event reaching val
        import bisect
        pos = bisect.bisect_left(evs, (val, -1))
        if pos == len(evs):
            return None
        return clocks[evs[pos][1]]

    class _EmptySI:
        on_wait = ()
        on_update = ()

    for idx, i in enumerate(insts):
        si = i.sync_info or _EmptySI
        c = {}
        p = proc_of(i, idx)
        if i.opcode != "Ldweights":
            prev = last_in_proc.get(p)
            if prev is not None:
                join(c, clocks[prev])
            pend = ldw_pending.pop(p, None)
            if pend is not None:
                join(c, pend)
            last_in_proc[p] = idx
        for w in si.on_wait:
            pc = producer_clock(w.ant_name, w.wait_value)
            if pc is not None:
                join(c, pc)
            if c.get(w.ant_name, -1) < w.wait_value:
                c[w.ant_name] = w.wait_value
        for u in si.on_update:
            sem = u.ant_name
            cum = sem_cum.get(sem, 0) + u.update_value
            sem_cum[sem] = cum
            sem_events.setdefault(sem, []).append((cum, idx))
            if c.get(sem, -1) < cum:
                c[sem] = cum
        clocks[idx] = c
        if i.opcode == "Ldweights":
            ldw_pending[p] = join(ldw_pending.get(p, {}), dict(c))

    # pruning pass, walking issue order per processor:
    #   (a) waits on the instruction's own processor semaphore (in-order
    #       completion within a processor),
    #   (b) waits transitively covered by another kept wait's causal past,
    #   (c) waits at-or-below what an earlier instruction on the same
    #       issue processor already waited for (semaphores are monotone).
    PRUNABLE = {
        "Matmult", "Ldweights", "DMACopy", "Activation", "TensorCopy",
        "TensorReduce", "TensorScalarPtr", "TensorTensor", "Memset",
        "Drain",
    }
    stripped = 0
    proc_hist = {}   # proc -> recent [(idx, inst, proc_sem_cum_after)]
    proc_sem = {}    # proc -> its completion semaphore name
    upd_cum = {}     # sem -> cumulative update value (pruning pass copy)
    # issue proc -> clock of everything provably completed before the
    # proc's current issue point (prior waits' targets AND their causal
    # pasts — a satisfied wait implies its producer's whole past, and
    # semaphores are monotone)
    observed = {}

    for idx, i in enumerate(insts):
        si = i.sync_info
        if si is None:
            continue
        p = proc_of(i, idx)
        obs = observed.setdefault(p, {})
        waits = list(si.on_wait)
        a_dropped = []
        if i.opcode in PRUNABLE and waits:
            eng = str(i.engine).split(".")[-1]
            kept = []
            for w in waits:
                sem_eng = w.ant_name.rsplit("_", 1)[0]
                # rule (a): same-engine completion is in program order, so a
                # wait on the engine's own semaphore is vacuous. NOT applied
                # to DMA self-queue waits: a queue's sem increments are only
                # ordered if the previous transfer provably completed, which
                # is rule (b)'s job. Dropped waits still hold at execution
                # time (FIFO engines execute in order), so they remain
                # usable as cover and observation.
                if i.opcode != "DMACopy" and sem_eng == eng:
                    a_dropped.append(w)
                    continue
                if obs.get(w.ant_name, -1) >= w.wait_value:
                    continue           # rule (c): already observed
                kept.append(w)
            # rule (b): transitive cover by other kept or (a)-dropped waits
            changed = True
            while changed and len(kept) > 1:
                changed = False
                for w in list(kept):
                    cover = {}
                    for x in kept + a_dropped:
                        if x is w:
                            continue
                        pc = producer_clock(x.ant_name, x.wait_value)
                        if pc is not None:
                            join(cover, pc)
                    if cover.get(w.ant_name, -1) >= w.wait_value:
                        kept.remove(w)
                        changed = True
            # fallback: migrate excess waits to an earlier same-proc
            # instruction with a free wait slot. Moving a wait earlier on
            # the issuing processor only strengthens ordering; it cannot
            # deadlock as long as the wait's producer does not causally
            # depend on the target instruction or anything after it on
            # this proc (checked via the producer's clock).
            while len(kept) > 1:
                placed = False
                for w in list(kept):
                    pcw = producer_clock(w.ant_name, w.wait_value) or {}
                    for t_idx, t_inst, t_cum in reversed(proc_hist.get(p, [])):
                        if t_inst.sync_info is None:
                            continue
                        psem = proc_sem.get(p)
                        if psem is not None and pcw.get(psem, -1) >= t_cum:
                            break  # producer needs this inst or later: stop
                        tw = list(t_inst.sync_info.on_wait)
                        if len(tw) == 0:
                            t_inst.sync_info.on_wait = [w]
                        elif len(tw) == 1 and tw[0].ant_name == w.ant_name:
                            if tw[0].wait_value < w.wait_value:
                                t_inst.sync_info.on_wait = [w]
                        else:
                            continue
                        kept.remove(w)
                        placed = True
                        break
                    if placed:
                        break
                if not placed:
                    break
            if len(kept) != len(waits):
                si.on_wait = kept
                stripped += 1
            waits = kept
        for w in list(waits) + a_dropped:
            if obs.get(w.ant_name, -1) < w.wait_value:
                obs[w.ant_name] = w.wait_value
            pc = producer_clock(w.ant_name, w.wait_value)
            if pc is not None:
                join(obs, pc)
        cum = None
        for u in (si.on_update or ()):
            sem_eng_u = u.ant_name.rsplit("_", 1)[0]
            if sem_eng_u == str(i.engine).split(".")[-1] or "DMA" in u.ant_name:
                proc_sem[p] = u.ant_name
                cum = upd_cum.get(u.ant_name, 0) + u.update_value
                upd_cum[u.ant_name] = cum
        proc_hist.setdefault(p, []).append(
            (idx, i, cum if cum is not None else upd_cum.get(proc_sem.get(p, ""), 0))
        )
        if len(proc_hist[p]) > 64:
            proc_hist[p] = proc_hist[p][-64:]
    return stripped


_NC_CACHE = {}


def _get_nc(plan):
    sig = plan.signature
    if sig not in _NC_CACHE:
        _NC_CACHE[sig] = _build_nc(
            plan.t_tiles, plan.ns, plan.chunks, plan.n_tot
        )
    return _NC_CACHE[sig]


# --------------------------------------------------------------------------
# Host-side data prep
# --------------------------------------------------------------------------

def _prepare_in_maps(plan, im_set, s_seq):
    im_set = np.asarray(im_set, dtype=np.float32)
    s_seq = np.asarray(s_seq, dtype=np.float32)
    im = im_set[:, 1:, :]                     # [B, LI, D]
    s = s_seq[:, 1:1 + LS, :]                 # [B, LS, D]

    # ---- packed image columns per column shard, chunk-contiguous -------
    # [128, KT*n_tot] fp8: per partition p, chunk ci occupies
    # [KT*coff : KT*(coff+ccols)] laid out k-major, c-minor, so each
    # chunk's DMA is 128 contiguous descriptors.
    n_tot = plan.n_tot
    im_flat = im.reshape(B * LI, D)
    im_pks = []
    floors = []
    for cs in range(N_CS):
        src = np.full(n_tot, -1, np.int64)    # flat (b*LI + i) or -1 pad
        for (ccols, coff, runs, s0, nsl) in plan.chunks:
            for (w, nb, roff, slot0) in runs:
                col = coff + roff
                for sb in range(slot0, slot0 + nb):
                    b = plan.shard_bs[cs][sb]
                    v = min(int(plan.im_l[b]), w)
                    src[col:col + v] = b * LI + np.arange(v)
                    col += w
        sel = np.zeros((n_tot, D), np.float32)
        valid = src >= 0
        sel[valid] = im_flat[src[valid]]
        sel8 = sel.astype(NP_F8)              # [n_tot, D]
        out = np.empty((128, KT * n_tot), NP_F8)
        for (ccols, coff, runs, s0, nsl) in plan.chunks:
            blk = sel8[coff:coff + ccols].reshape(ccols, KT, 128)
            out[:, KT * coff:KT * (coff + ccols)] = (
                blk.transpose(2, 1, 0).reshape(128, KT * ccols)
            )
        im_pks.append(np.ascontiguousarray(out))
        bs = np.array(plan.shard_bs[cs])
        floor_row = np.where(plan.im_l[bs] < LI, 0.0, -60000.0)
        floors.append(
            np.broadcast_to(floor_row.astype(np.float16)[None, :], (128, BL))
            .copy()
        )

    # ---- per-word-shard packed sentences + indicators ------------------
    T = plan.t_tiles
    ns = plan.ns
    rows_cap = T * 128
    s_flat = s.reshape(B * LS, D)
    sTs = []
    inds = []
    for ws in range(N_WS):
        sents = plan.core_sents[ws]
        rows = []
        ind = np.zeros((rows_cap, ns), np.float16)
        r = 0
        for slot, t in enumerate(sents):
            sl = int(plan.s_l[t])
            rows.append(t * LS + np.arange(sl))
            ind[r:r + sl, slot] = 1.0
            r += sl
        rows = np.concatenate(rows) if rows else np.zeros(0, np.int64)
        sel_s = np.zeros((rows_cap, D), np.float32)
        sel_s[:len(rows)] = s_flat[rows]
        # sT[p, tau, k, m] = sel_s[tau*128 + m, k*128 + p]
        sTs.append(
            np.ascontiguousarray(
                sel_s.astype(NP_F8)
                .reshape(T, 128, KT, 128).transpose(3, 0, 2, 1)
            ).reshape(128, T * KT * 128)
        )
        # ind tile layout [m, tau, slot]
        inds.append(
            np.ascontiguousarray(
                ind.reshape(T, 128, ns).transpose(1, 0, 2)
            ).reshape(128, T * ns)
        )

    return [
        {
            "s_t": sTs[c // N_CS],
            "im_pk": im_pks[c % N_CS],
        }
        for c in range(NCORES)
    ]


def _loss_from_cores(plan, core_outs):
    T = plan.t_tiles
    aggr = np.zeros((B, B), np.float64)
    for c in range(NCORES):
        ws, cs = c // N_CS, c % N_CS
        # [128, T*BL] fp16 -> word-row-major [T*128, BL]
        m = (
            np.asarray(core_outs[c])
            .reshape(128, T, BL).transpose(1, 0, 2)
            .reshape(T * 128, BL).astype(np.float64)
        )
        bs = np.array(plan.shard_bs[cs])
        floor_row = np.where(plan.im_l[bs] < LI, 0.0, -np.inf)
        m = np.maximum(m, floor_row[None, :])
        r = 0
        for t in plan.core_sents[ws]:
            sl = int(plan.s_l[t])
            aggr[bs, t] = m[r:r + sl].sum(axis=0)
            r += sl
    diag = np.diag(aggr)
    cost_s = MARGIN + aggr - diag[:, None]
    cost_im = MARGIN + aggr - diag[None, :]
    np.fill_diagonal(cost_s, 0.0)
    np.fill_diagonal(cost_im, 0.0)
    cost_s = np.maximum(cost_s, 0.0)
    cost_im = np.maximum(cost_im, 0.0)
    loss = cost_s.max(axis=1).sum() + cost_im.max(axis=0).sum()
    return np.array(loss, dtype=np.float32)


def _run(im_set, s_seq, im_len, s_len, **spmd_kwargs):
    plan = _make_plan(im_len, s_len)
    nc = _get_nc(plan)
    in_maps = _prepare_in_maps(plan, im_set, s_seq)
    res = run_bass_kernel_spmd(
        nc, in_maps, core_ids=list(range(NCORES)), **spmd_kwargs
    )
    loss = _loss_from_cores(plan, [r["m_out"] for r in res.results])
    return loss, res


def kernel(im_set, s_seq, im_len, s_len):
    loss, _ = _run(im_set, s_seq, im_len, s_len)
    return loss


def _install_ntff_hook_shim():
    """This image's antenv lacks axon_hooks; recreate it from trn_boot's
    ctypes path so run_bass_kernel_spmd(trace=True) can capture NTFFs."""
    import sys
    import types

    if "antenv.axon_hooks" in sys.modules:
        return
    from trn_agent_boot.trn_boot import _ntff_profile_via_ctypes

    hook = _ntff_profile_via_ctypes("/opt/axon/libaxon_pjrt.so")
    mod = types.ModuleType("antenv.axon_hooks")
    mod._hook = hook
    mod.get_axon_ntff_profile_hook = lambda: mod._hook
    mod.set_axon_ntff_profile_hook = lambda h: setattr(mod, "_hook", h)
    sys.modules["antenv.axon_hooks"] = mod
    import antenv

    antenv.axon_hooks = mod


def kernel_traced(im_set, s_seq, im_len, s_len, **kwargs):
    """Returns (loss, BassKernelResults-with-exec_time_ns)."""
    _install_ntff_hook_shim()
    loss, res = _run(im_set, s_seq, im_len, s_len, trace=True, **kwargs)
    return loss, res


# revision 26
# speedup vs baseline: 1.1771x; 1.1771x over previous
"""AlignmentContrastiveLoss Trainium2 kernel (v10).

Math (matching the reference):
  im = im_set[:, 1:, :]        -> [128, 64, 1024]  rows (b, i)
  s  = s_seq[:, 1:-2, :]       -> [128, 64, 1024]  rows (t, j)
  align[b,t,i,j] = im[b,i,:] . s[t,j,:]   (masked entries forced to 0)
  aggr[b,t] = sum_j max_i align
  loss = hinge-contrastive reduction of aggr [128,128]

Structure:
  - Work split 4 word-shards x 2 image-column-shards across 8 cores.
    Valid words only (bin-packed); valid image regions only, padded per
    slot to a shared two-shard template width (multiple of 4).
  - fp8e4 DoubleRow matmuls (contraction 256/instruction, ~157 TF/s).
  - Chunk-contiguous HBM layout: each PSUM-chunk's [KT x ccols] block is
    contiguous per partition, so a chunk DMA is 128 descriptors of up to
    4KB instead of 1024 x ~500B.
  - All DMAs are issued up front on the SP HWDGE queue in priority
    order, EXCEPT sT0 and sT1 which ride the Act queue so the first two
    tau slices transfer in parallel with the first image chunks (the
    16 DMA engines split across active queues, so only this short
    startup prefix is dual-queue). The first two chunks transfer in
    ko-halves so the first matmuls gate on quarter-size pieces.
  - Tau-major over two chunk groups: group 0 = the two smallest chunks
    (starts as soon as ~250KB has landed), group 1 = the rest. Within a
    tau, each chunk runs its 4 ko-matmuls back-to-back and immediately
    emits its segmented max-reduces, so each chunk's PSUM stop lands as
    early as possible and DVE pipelines behind the PE.
  - The device outputs the per-word-row maxes m_all[128, T*BL] (fp16)
    straight from the last reduce; the masked-region floor clamp, the
    j-sum over word rows, and the hinge loss all run on the host. This
    removes the indicator matmuls, clamps, PSUM accumulation bank, and
    final copy from the device critical path.
  - The PE is power-throttled on sustained fp8 work (~50% util-limit
    windows, 7-10us total), so junk warmup matmuls and extra padding
    columns are counter-productive - the schedule minimizes total PE
    cycles and lets the PE idle before data arrives rather than burning
    the power budget.
  - A post-Tile pass prunes/migrates redundant semaphore waits (TPB ISA
    encodes ONE wait per instruction); see _prune_redundant_waits.

Compiled programs are cached per plan signature.
"""

import numpy as np
import ml_dtypes

import concourse.bass as bass
import concourse.mybir as mybir
import concourse.tile as tile
from concourse.bass_utils import run_bass_kernel_spmd

F32 = mybir.dt.float32
F16 = mybir.dt.float16
F8 = mybir.dt.float8e4
NP_F8 = ml_dtypes.float8_e4m3

MARGIN = 0.2
B = 128
LI = 64          # image regions after slicing
LS = 64          # words after slicing
D = 1024
KT = D // 128    # 8 contraction subtiles of 128
NCORES = 8
CHUNK_MAX = 512  # PSUM bank width (fp32)
DR = mybir.MatmulPerfMode.DoubleRow

N_WS = 4    # word (sentence) shards
N_CS = 2    # image-column shards; core id = ws * N_CS + cs
BL = B // N_CS  # b's per column shard

N_WARM = 0       # junk warmup matmuls burn power budget -> more PE throttling
POOL_REDUCE = 0  # gpsimd tensor_reduce can't do free-dim (X) reduces on TRN2


class _Plan:
    __slots__ = (
        "im_l", "s_l", "shard_bs", "template", "chunks", "n_tot",
        "t_tiles", "ns", "core_sents", "signature",
    )


def _make_plan(im_len, s_len):
    p = _Plan()
    im_l = np.asarray(im_len).astype(np.int64) - 1
    s_l = np.asarray(s_len).astype(np.int64) - 3
    im_l = np.clip(im_l, 0, LI)
    s_l = np.clip(s_l, 0, LS)
    p.im_l = im_l
    p.s_l = s_l

    # --- image columns ---------------------------------------------------
    # width = valid count padded up to a multiple of 4. No zero-guard
    # column: the reference's "max includes 0 when any i is masked" is
    # reproduced by a post-reduce clamp against a per-slot floor.
    widths = np.maximum(4, 4 * ((im_l + 3) // 4)).astype(np.int64)
    order = np.argsort(-widths, kind="stable")
    shard_bs = [
        [int(order[j * N_CS + k]) for j in range(BL)] for k in range(N_CS)
    ]
    template = [int(widths[order[j * N_CS]]) for j in range(BL)]
    p.shard_bs = shard_bs
    p.template = template

    # chunks over the template (shared by all cores): greedy <= CHUNK_MAX
    # columns, breaks only at slot boundaries; runs = maximal equal-width
    # slot ranges for the segmented reduces.
    chunks = []  # [cols, col_off, runs, slot0, nslots]
    i = 0
    while i < BL:
        cols = 0
        runs = []
        s0 = i
        while i < BL:
            w = template[i]
            if cols + w > CHUNK_MAX:
                break
            j = i
            while (
                j < BL
                and template[j] == w
                and cols + (j - i + 1) * w <= CHUNK_MAX
            ):
                j += 1
            runs.append((w, j - i, cols, i))
            cols += (j - i) * w
            i = j
        chunks.append([cols, 0, tuple(runs), s0, i - s0])
    # traversal order: the two smallest chunks first (group 0 — they gate
    # the first real matmuls), then the rest descending with the smallest
    # of them last so the final reduce+j-sum tail is short.
    chunks.sort(key=lambda c: c[0])
    if len(chunks) > 3:
        head, rest = chunks[:2], chunks[2:]
        tail = rest.pop(0)  # smallest of rest
        chunks = head + rest[::-1] + [tail]
    col = 0
    for c in chunks:
        c[1] = col
        col += c[0]
    chunks = [tuple(c) for c in chunks]
    p.chunks = chunks
    p.n_tot = col

    # --- sentence packing: greedy bin-pack into N_WS word shards --------
    order_s = np.argsort(-s_l, kind="stable")
    loads = [0] * N_WS
    core_sents = [[] for _ in range(N_WS)]
    for t in order_s:
        c = int(np.argmin(loads))
        core_sents[c].append(int(t))
        loads[c] += int(s_l[t])
    p.core_sents = core_sents
    p.t_tiles = max(1, int(-(-max(loads) // 128)))
    max_ns = max(len(cs) for cs in core_sents)
    p.ns = -(-max_ns // 8) * 8  # pad to multiple of 8

    p.signature = (
        p.t_tiles, p.ns,
        tuple((c[0], c[1], c[3], c[4]) for c in chunks),
    )
    return p


# --------------------------------------------------------------------------
# Device program
# --------------------------------------------------------------------------

def _build_nc(t_tiles, ns, chunks, n_tot,
              prune=True, detect_races=True,
              n_warm=N_WARM, pool_reduce=POOL_REDUCE):
    from contextlib import ExitStack

    T = t_tiles
    nc = bass.Bass(detect_race_conditions=detect_races)
    sT_in = nc.dram_tensor("s_t", [128, T * KT * 128], F8, kind="ExternalInput")
    im_in = nc.dram_tensor("im_pk", [128, KT * n_tot], F8, kind="ExternalInput")
    m_out = nc.dram_tensor("m_out", [128, T * BL], F16, kind="ExternalOutput")

    NCH = len(chunks)
    with tile.TileContext(nc) as tc, ExitStack() as ctx:
        consts = ctx.enter_context(tc.tile_pool(name="consts", bufs=1))
        mp = ctx.enter_context(tc.tile_pool(name="mp", bufs=1))
        imtp = ctx.enter_context(tc.tile_pool(name="imtp", bufs=NCH))
        outp = ctx.enter_context(tc.tile_pool(name="outp", bufs=1))
        psm = ctx.enter_context(tc.tile_pool(name="psm", bufs=7, space="PSUM"))

        # sT[p, tau, k, m] = s_packed[tau*128 + m, k*128 + p]  (fp8)
        sT = consts.tile([128, T, KT, 128], F8)
        # zero tile feeding the warmup matmuls
        zw = consts.tile([128, 2, 256], F8)

        # m_all[p, tau, slot] = max_i of align for word row (tau, p) vs
        # this column shard's slot-th image batch. Clamping against the
        # masked-region floor and the j-sum over word rows both happen on
        # the host - m_all is the kernel's output.
        m_all = mp.tile([128, T, BL], F16)

        # ---- PE warmup: junk matmuls on zeroed SBUF burn the p-state
        # ramp while the first real DMAs are still in flight.
        if n_warm:
            nc.gpsimd.memset(zw[:], 0.0)
            warm = psm.tile([128, 256], F32, tag="pm", name="warm")
            for _ in range(n_warm):
                nc.tensor.matmul(
                    warm[:], zw[:, :, 0:128], zw[:],
                    start=True, stop=True, perf_mode=DR,
                    skip_group_check=True,
                )

        # ---- all DMAs issued up front on a single HWDGE queue (SP), in
        # strict priority order: splitting across two queues halves each
        # queue's share of the 16 DMA engines and delays the startup-
        # critical transfers. The first two chunks are transferred in
        # ko-halves so the first matmuls gate on quarter-size pieces.
        imts = [None] * NCH

        def _issue_chunk(ci, halves=False):
            ccols, coff, runs, s0, nsl = chunks[ci]
            imt = imtp.tile([128, KT, ccols], F8, tag="imt", name=f"im{ci}")
            imts[ci] = imt
            if halves:
                kh = KT // 2
                nc.sync.dma_start(
                    imt[:, 0:kh],
                    im_in[:, KT * coff:KT * coff + kh * ccols].rearrange(
                        "p (k c) -> p k c", k=kh
                    ),
                )
                nc.sync.dma_start(
                    imt[:, kh:KT],
                    im_in[:, KT * coff + kh * ccols:KT * (coff + ccols)]
                    .rearrange("p (k c) -> p k c", k=kh),
                )
            else:
                nc.sync.dma_start(
                    imt[:],
                    im_in[:, KT * coff:KT * (coff + ccols)].rearrange(
                        "p (k c) -> p k c", k=KT
                    ),
                )

        def _issue_sT(ta, tb):
            nc.sync.dma_start(
                sT[:, ta:tb],
                sT_in[:, ta * KT * 128:tb * KT * 128],
            )

        # sT0 rides the Act queue so it transfers in parallel with c0 on
        # the SP queue; Act goes idle right after, returning its DMA
        # engines to the SP queue for the rest of the stream.
        nc.scalar.dma_start(sT[:, 0:1], sT_in[:, 0:KT * 128])
        _issue_chunk(0, halves=True)
        if T > 1:
            nc.scalar.dma_start(sT[:, 1:2], sT_in[:, KT * 128:2 * KT * 128])
        _issue_chunk(1, halves=True)
        _issue_sT(2, min(3, T))
        if T > 3:
            _issue_sT(3, min(6, T))
        if NCH > 2:
            _issue_chunk(2)
        if T > 6:
            _issue_sT(6, T)
        for ci in range(3, NCH):
            _issue_chunk(ci)

        # ---- tau-major main loop over two chunk groups.
        groups = [(0, min(2, NCH)), (min(2, NCH), NCH)]
        groups = [(a, b) for a, b in groups if b > a]

        for gi, (clo, chi) in enumerate(groups):
            last_group = gi == len(groups) - 1
            for tau in range(T):
                # per chunk: 4 ko-inner matmuls, then that chunk's
                # reduces — each chunk's PSUM stop lands as early as
                # possible so DVE pipelines behind the PE instead of all
                # reduces bunching after the tau's last matmul.
                for ci in range(clo, chi):
                    ccols, coff, runs, s0, nsl = chunks[ci]
                    pt = psm.tile(
                        [128, ccols], F32, tag="pm", name=f"pm{ci}_{tau}"
                    )
                    for ko in range(KT // 2):
                        nc.tensor.matmul(
                            pt[:],
                            sT[:, tau, 2 * ko:2 * ko + 2, :],
                            imts[ci][:, 2 * ko:2 * ko + 2, :],
                            start=(ko == 0),
                            stop=(ko == KT // 2 - 1),
                            perf_mode=DR,
                        )
                    for (w, nb, roff, slot0) in runs:
                        nc.vector.reduce_max(
                            m_all[:, tau, slot0:slot0 + nb],
                            pt[:, roff:roff + w * nb].rearrange(
                                "p (n w) -> p n w", w=w
                            ),
                            axis=mybir.AxisListType.X,
                        )
        nc.sync.dma_start(
            m_out[:], m_all[:].rearrange("p t s -> p (t s)")
        )

    if prune:
        _prune_redundant_waits(nc)
    return nc


def _prune_redundant_waits(nc):
    """Drop semaphore waits that are provably redundant on the final schedule.

    Walrus's per-instruction ISA structs encode very few sync waits (one for
    PE Matmult / HWDGE DMA), and Tile's wait placement leaves redundant ones:
    (a) waits on the instruction's own processor semaphore (PE matmuls
    complete in program order; a HWDGE queue executes its descriptors FIFO),
    and (b) waits whose target completion is already in the causal past of
    another wait kept on the same instruction. Both classes are dropped here
    using a conservative happens-before computed from the untouched program.

    "Processor" is the engine, except DMACopy where it is the HW queue
    (identified by its update semaphore). Ldweights can be pulled ahead of
    in-flight matmuls by the PE, so it neither extends nor inherits the
    same-proc completion chain.
    """
    insts = []
    for f in nc.m.functions:
        for bb in f.blocks:
            insts.extend(bb.instructions)

    def proc_of(i, idx):
        if i.opcode == "DMACopy":
            ups = i.sync_info.on_update
            qs = [u.ant_name for u in ups if "DMA" in u.ant_name]
            if len(qs) == 1:
                return qs[0]
            return f"__solo_{idx}"
        return f"__eng_{i.engine}"

    # completion clocks: clock[i] = {sem: min guaranteed value when i completes}
    sem_events = {}   # sem -> list of (cumval, inst_idx) in inc order
    sem_cum = {}
    clocks = [None] * len(insts)
    last_in_proc = {}
    # Ldweights waits are satisfied before any later instruction on the
    # engine dispatches (NX evaluates waits in program order; the PE can
    # only pull an LDW *earlier*), so they propagate forward — but LDW
    # itself must not inherit the chain (it may run before prior MMs
    # complete).
    ldw_pending = {}

    def join(a, b):
        for k, v in b.items():
            if a.get(k, -1) < v:
                a[k] = v
        return a

    def producer_clock(sem, val):
        evs = sem_events.get(sem)
        if not evs:
            return None
        # first event reaching val
        import bisect
        pos = bisect.bisect_left(evs, (val, -1))
        if pos == len(evs):
            return None
        return clocks[evs[pos][1]]

    class _EmptySI:
        on_wait = ()
        on_update = ()

    for idx, i in enumerate(insts):
        si = i.sync_info or _EmptySI
        c = {}
        p = proc_of(i, idx)
        if i.opcode != "Ldweights":
            prev = last_in_proc.get(p)
            if prev is not None:
                join(c, clocks[prev])
            pend = ldw_pending.pop(p, None)
            if pend is not None:
                join(c, pend)
            last_in_proc[p] = idx
        for w in si.on_wait:
            pc = producer_clock(w.ant_name, w.wait_value)
            if pc is not None:
                join(c, pc)
            if c.get(w.ant_name, -1) < w.wait_value:
                c[w.ant_name] = w.wait_value
        for u in si.on_update:
            sem = u.ant_name
            cum = sem_cum.get(sem, 0) + u.update_value
            sem_cum[sem] = cum
            sem_events.setdefault(sem, []).append((cum, idx))
            if c.get(sem, -1) < cum:
                c[sem] = cum
        clocks[idx] = c
        if i.opcode == "Ldweights":
            ldw_pending[p] = join(ldw_pending.get(p, {}), dict(c))

    # pruning pass, walking issue order per processor:
    #   (a) waits on the instruction's own processor semaphore (in-order
    #       completion within a processor),
    #   (b) waits transitively covered by another kept wait's causal past,
    #   (c) waits at-or-below what an earlier instruction on the same
    #       issue processor already waited for (semaphores are monotone).
    PRUNABLE = {
        "Matmult", "Ldweights", "DMACopy", "Activation", "TensorCopy",
        "TensorReduce", "TensorScalarPtr", "TensorTensor", "Memset",
        "Drain",
    }
    stripped = 0
    proc_hist = {}   # proc -> recent [(idx, inst, proc_sem_cum_after)]
    proc_sem = {}    # proc -> its completion semaphore name
    upd_cum = {}     # sem -> cumulative update value (pruning pass copy)
    # issue proc -> clock of everything provably completed before the
    # proc's current issue point (prior waits' targets AND their causal
    # pasts — a satisfied wait implies its producer's whole past, and
    # semaphores are monotone)
    observed = {}

    for idx, i in enumerate(insts):
        si = i.sync_info
        if si is None:
            continue
        p = proc_of(i, idx)
        obs = observed.setdefault(p, {})
        waits = list(si.on_wait)
        a_dropped = []
        if i.opcode in PRUNABLE and waits:
            eng = str(i.engine).split(".")[-1]
            kept = []
            for w in waits:
                sem_eng = w.ant_name.rsplit("_", 1)[0]
                # rule (a): same-engine completion is in program order, so a
                # wait on the engine's own semaphore is vacuous. NOT applied
                # to DMA self-queue waits: a queue's sem increments are only
                # ordered if the previous transfer provably completed, which
                # is rule (b)'s job. Dropped waits still hold at execution
                # time (FIFO engines execute in order), so they remain
                # usable as cover and observation.
                if i.opcode != "DMACopy" and sem_eng == eng:
                    a_dropped.append(w)
                    continue
                if obs.get(w.ant_name, -1) >= w.wait_value:
                    continue           # rule (c): already observed
                kept.append(w)
            # rule (b): transitive cover by other kept or (a)-dropped waits
            changed = True
            while changed and len(kept) > 1:
                changed = False
                for w in list(kept):
                    cover = {}
                    for x in kept + a_dropped:
                        if x is w:
                            continue
                        pc = producer_clock(x.ant_name, x.wait_value)
                        if pc is not None:
                            join(cover, pc)
                    if cover.get(w.ant_name, -1) >= w.wait_value:
                        kept.remove(w)
                        changed = True
            # fallback: migrate excess waits to an earlier same-proc
            # instruction with a free wait slot. Moving a wait earlier on
            # the issuing processor only strengthens ordering; it cannot
            # deadlock as long as the wait's producer does not causally
            # depend on the target instruction or anything after it on
            # this proc (checked via the producer's clock).
            while len(kept) > 1:
                placed = False
                for w in list(kept):
                    pcw = producer_clock(w.ant_name, w.wait_value) or {}
                    for t_idx, t_inst, t_cum in reversed(proc_hist.get(p, [])):
                        if t_inst.sync_info is None:
                            continue
                        psem = proc_sem.get(p)
                        if psem is not None and pcw.get(psem, -1) >= t_cum:
                            break  # producer needs this inst or later: stop
                        tw = list(t_inst.sync_info.on_wait)
                        if len(tw) == 0:
                            t_inst.sync_info.on_wait = [w]
                        elif len(tw) == 1 and tw[0].ant_name == w.ant_name:
                            if tw[0].wait_value < w.wait_value:
                                t_inst.sync_info.on_wait = [w]
                        else:
                            continue
                        kept.remove(w)
                        placed = True
                        break
                    if placed:
                        break
                if not placed:
                    break
            if len(kept) != len(waits):
                si.on_wait = kept
                stripped += 1
            waits = kept
        for w in list(waits) + a_dropped:
            if obs.get(w.ant_name, -1) < w.wait_value:
                obs[w.ant_name] = w.wait_value
            pc = producer_clock(w.ant_name, w.wait_value)
            if pc is not None:
                join(obs, pc)
        cum = None
        for u in (si.on_update or ()):
            sem_eng_u = u.ant_name.rsplit("_", 1)[0]
            if sem_eng_u == str(i.engine).split(".")[-1] or "DMA" in u.ant_name:
                proc_sem[p] = u.ant_name
                cum = upd_cum.get(u.ant_name, 0) + u.update_value
                upd_cum[u.ant_name] = cum
        proc_hist.setdefault(p, []).append(
            (idx, i, cum if cum is not None else upd_cum.get(proc_sem.get(p, ""), 0))
        )
        if len(proc_hist[p]) > 64:
            proc_hist[p] = proc_hist[p][-64:]
    return stripped


_NC_CACHE = {}


def _get_nc(plan):
    sig = plan.signature
    if sig not in _NC_CACHE:
        _NC_CACHE[sig] = _build_nc(
            plan.t_tiles, plan.ns, plan.chunks, plan.n_tot
        )
    return _NC_CACHE[sig]


# --------------------------------------------------------------------------
# Host-side data prep
# --------------------------------------------------------------------------

def _prepare_in_maps(plan, im_set, s_seq):
    im_set = np.asarray(im_set, dtype=np.float32)
    s_seq = np.asarray(s_seq, dtype=np.float32)
    im = im_set[:, 1:, :]                     # [B, LI, D]
    s = s_seq[:, 1:1 + LS, :]                 # [B, LS, D]

    # ---- packed image columns per column shard, chunk-contiguous -------
    # [128, KT*n_tot] fp8: per partition p, chunk ci occupies
    # [KT*coff : KT*(coff+ccols)] laid out k-major, c-minor, so each
    # chunk's DMA is 128 contiguous descriptors.
    n_tot = plan.n_tot
    im_flat = im.reshape(B * LI, D)
    im_pks = []
    floors = []
    for cs in range(N_CS):
        src = np.full(n_tot, -1, np.int64)    # flat (b*LI + i) or -1 pad
        for (ccols, coff, runs, s0, nsl) in plan.chunks:
            for (w, nb, roff, slot0) in runs:
                col = coff + roff
                for sb in range(slot0, slot0 + nb):
                    b = plan.shard_bs[cs][sb]
                    v = min(int(plan.im_l[b]), w)
                    src[col:col + v] = b * LI + np.arange(v)
                    col += w
        sel = np.zeros((n_tot, D), np.float32)
        valid = src >= 0
        sel[valid] = im_flat[src[valid]]
        sel8 = sel.astype(NP_F8)              # [n_tot, D]
        out = np.empty((128, KT * n_tot), NP_F8)
        for (ccols, coff, runs, s0, nsl) in plan.chunks:
            blk = sel8[coff:coff + ccols].reshape(ccols, KT, 128)
            out[:, KT * coff:KT * (coff + ccols)] = (
                blk.transpose(2, 1, 0).reshape(128, KT * ccols)
            )
        im_pks.append(np.ascontiguousarray(out))
        bs = np.array(plan.shard_bs[cs])
        floor_row = np.where(plan.im_l[bs] < LI, 0.0, -60000.0)
        floors.append(
            np.broadcast_to(floor_row.astype(np.float16)[None, :], (128, BL))
            .copy()
        )

    # ---- per-word-shard packed sentences + indicators ------------------
    T = plan.t_tiles
    ns = plan.ns
    rows_cap = T * 128
    s_flat = s.reshape(B * LS, D)
    sTs = []
    inds = []
    for ws in range(N_WS):
        sents = plan.core_sents[ws]
        rows = []
        ind = np.zeros((rows_cap, ns), np.float16)
        r = 0
        for slot, t in enumerate(sents):
            sl = int(plan.s_l[t])
            rows.append(t * LS + np.arange(sl))
            ind[r:r + sl, slot] = 1.0
            r += sl
        rows = np.concatenate(rows) if rows else np.zeros(0, np.int64)
        sel_s = np.zeros((rows_cap, D), np.float32)
        sel_s[:len(rows)] = s_flat[rows]
        # sT[p, tau, k, m] = sel_s[tau*128 + m, k*128 + p]
        sTs.append(
            np.ascontiguousarray(
                sel_s.astype(NP_F8)
                .reshape(T, 128, KT, 128).transpose(3, 0, 2, 1)
            ).reshape(128, T * KT * 128)
        )
        # ind tile layout [m, tau, slot]
        inds.append(
            np.ascontiguousarray(
                ind.reshape(T, 128, ns).transpose(1, 0, 2)
            ).reshape(128, T * ns)
        )

    return [
        {
            "s_t": sTs[c // N_CS],
            "im_pk": im_pks[c % N_CS],
        }
        for c in range(NCORES)
    ]


def _loss_from_cores(plan, core_outs):
    T = plan.t_tiles
    aggr = np.zeros((B, B), np.float64)
    for c in range(NCORES):
        ws, cs = c // N_CS, c % N_CS
        # [128, T*BL] fp16 -> word-row-major [T*128, BL]
        m = (
            np.asarray(core_outs[c])
            .reshape(128, T, BL).transpose(1, 0, 2)
            .reshape(T * 128, BL).astype(np.float64)
        )
        bs = np.array(plan.shard_bs[cs])
        floor_row = np.where(plan.im_l[bs] < LI, 0.0, -np.inf)
        m = np.maximum(m, floor_row[None, :])
        r = 0
        for t in plan.core_sents[ws]:
            sl = int(plan.s_l[t])
            aggr[bs, t] = m[r:r + sl].sum(axis=0)
            r += sl
    diag = np.diag(aggr)
    cost_s = MARGIN + aggr - diag[:, None]
    cost_im = MARGIN + aggr - diag[None, :]
    np.fill_diagonal(cost_s, 0.0)
    np.fill_diagonal(cost_im, 0.0)
    cost_s = np.maximum(cost_s, 0.0)
    cost_im = np.maximum(cost_im, 0.0)
    loss = cost_s.max(axis=1).sum() + cost_im.max(axis=0).sum()
    return np.array(loss, dtype=np.float32)


def _run(im_set, s_seq, im_len, s_len, **spmd_kwargs):
    plan = _make_plan(im_len, s_len)
    nc = _get_nc(plan)
    in_maps = _prepare_in_maps(plan, im_set, s_seq)
    res = run_bass_kernel_spmd(
        nc, in_maps, core_ids=list(range(NCORES)), **spmd_kwargs
    )
    loss = _loss_from_cores(plan, [r["m_out"] for r in res.results])
    return loss, res


def kernel(im_set, s_seq, im_len, s_len):
    loss, _ = _run(im_set, s_seq, im_len, s_len)
    return loss


def _install_ntff_hook_shim():
    """This image's antenv lacks axon_hooks; recreate it from trn_boot's
    ctypes path so run_bass_kernel_spmd(trace=True) can capture NTFFs."""
    import sys
    import types

    if "antenv.axon_hooks" in sys.modules:
        return
    from trn_agent_boot.trn_boot import _ntff_profile_via_ctypes

    hook = _ntff_profile_via_ctypes("/opt/axon/libaxon_pjrt.so")
    mod = types.ModuleType("antenv.axon_hooks")
    mod._hook = hook
    mod.get_axon_ntff_profile_hook = lambda: mod._hook
    mod.set_axon_ntff_profile_hook = lambda h: setattr(mod, "_hook", h)
    sys.modules["antenv.axon_hooks"] = mod
    import antenv

    antenv.axon_hooks = mod


def kernel_traced(im_set, s_seq, im_len, s_len, **kwargs):
    """Returns (loss, BassKernelResults-with-exec_time_ns)."""
    _install_ntff_hook_shim()
    loss, res = _run(im_set, s_seq, im_len, s_len, trace=True, **kwargs)
    return loss, res
